# revision 41
# baseline (speedup 1.0000x reference)
"""Trainium2 Bass kernel for nn_InnerProductDecoder.

For each graph b: out[b] = P_b @ P_b^T where P_b is the zero-padded
[max_n, D] node-feature matrix of graph b (pad_sequence equivalent).

Strategy: data parallel over B (64 graphs / 8 cores). Graphs are
sorted by length and dealt round-robin so slot s on every core holds
a graph of length <= slot_len[s]; the SPMD program is built for the
slot-length profile, so each core only moves/computes its graphs'
ragged extents.

Design notes:
  * fp16 end-to-end on device (PSUM accumulation fp32): halves HBM
    traffic vs f32 and runs the PE at full rate. Host casts back.
    Measured rel err ~3e-4 against the f32 reference.
  * out[b] is symmetric: only upper-triangular 128-row blocks are
    computed/copied/stored (block m covers cols [128m, L)); host
    mirrors. ~37% less PE/copy/output-DMA work.
  * the critical path is the HBM wire (5.03 MB at ~330 GB/s) plus the
    PE stream that tracks it; everything else is arranged so no engine
    ever delays either stream:
      - one DMA queue tops out at ~273 GB/s; the full wire (~390) needs
        2-3 queues drawing descriptors concurrently. Input chunks are
        therefore dealt ROUND-ROBIN over the SP, ACT and GpSimd queues
        (chunk i -> queue i%3): the queues drain in lockstep, so bytes
        still arrive ~in PE need order. (Giving each queue a contiguous
        slot range instead is ~6us slower than one queue: slot-7 bytes
        steal wire from the slot-0 chunk the PE is stalled on.)
      - output DMAs are split between the SP and GpSimd queues (even /
        odd slots), so output write traffic co-flows with the input
        stream instead of serializing after it, and ACT stays a pure
        copy engine (its per-slot out-DMA issue used to starve the
        copy stream).
      - PE runs ~8 junk matmuls at block entry while the first input
        chunk is still on the wire: keeps the DVFS clock ramping
        (a cold PE runs ~half rate for ~3us after any idle gap).
  * PSUM->SBUF copies split between DVE and ACT (the only engines
    with PSUM read ports), balanced by estimated ns; ACT's
    activation-table load is triggered off-path before its copies.
  * NO buffer recycling: the whole per-core input (25.6KB/partition)
    and output (14KB/partition) live in SBUF at once, so every input
    DMA issues immediately and no engine ever waits on buffer reuse.
  * the Block-exit all-engine barrier is skipped (engine branches
    emitted manually): the runtime's end-of-program epilogue (an
    all-engine barrier + ~50 serial sem clears per engine) synchronizes
    engines anyway, so early finishers overlap their epilogue with the
    output tail.
  * tail: GpSimd alone waits final sem values then dma_reset +
    sem_clear (re-execution safety; NRT does not clear sems).

Per-core raw-Bass pipeline:

  sync  (SP) : input chunks 0,3,6,9; output DMAs for even slots
  scalar(ACT): input chunks 1,4,7,10 + act-table trigger; copy share
  gpsimd     : input chunks 2,5,8; output DMAs for odd slots; tail reset
  vector(DVE): its share of PSUM->SBUF copies
  tensor(PE) : 8 warmup matmuls, then 4*mb fp16 matmuls -> PSUM (x2 sets)
"""

import numpy as np

N_CORES = 8
B = 64
MAXN = 512
D = 512
PER_CORE = B // N_CORES  # 8 slots per core
KCH = D // 128  # 4 contraction chunks
N_WARMUP = 8  # PE clock-ramp matmuls at block entry

_prog_cache = {}


def _mb(l):
    return (l + 127) // 128


def _slot_blocks(L):
    """Upper-triangular block list for one slot: (m, W, rows, t) where the
    copy of block m lands at out cols [t, t+W) of the slot's region."""
    bl, t = [], 0
    for m in range(_mb(L)):
        W = L - 128 * m
        rows = min(128, W)
        bl.append((m, W, rows, t))
        t += W
    return bl


def _build_program(slot_lens):
    import concourse.bass as bass
    from concourse import mybir

    f32 = mybir.dt.float32
    f16 = mybir.dt.float16
    nc = bass.Bass()

    J = len(slot_lens)
    blocks = [_slot_blocks(L) for L in slot_lens]
    T = [bl[-1][3] + bl[-1][1] for bl in blocks]  # out width per slot
    in_off = np.concatenate([[0], np.cumsum([KCH * l for l in slot_lens])])
    # slot offsets (and the total row stride) padded to 128 fp16 elems so
    # every output-DMA descriptor lands on a 256B DRAM boundary (the
    # natural 13696B stride leaves half the write bursts straddling).
    Tp = [(t + 127) // 128 * 128 for t in T]
    out_off = np.concatenate([[0], np.cumsum(Tp)])
    # m-blocks completed after slot j (prefix sums for sem values)
    cum_mb = np.concatenate([[0], np.cumsum([len(bl) for bl in blocks])])

    # input DMA chunking (k-chunk ranges per slot): slots 0-1 arrive in
    # pieces so the PE can start on partial data; later slots whole.
    in_chunks = [[(0, 1), (1, 2), (2, KCH)], [(0, 2), (2, KCH)]] + [
        [(0, KCH)]
    ] * (J - 2)
    # chunk index before slot j (global chunk numbering = sem index)
    in_base = np.concatenate([[0], np.cumsum([len(c) for c in in_chunks])])
    n_chunks = int(in_base[J])

    xt = nc.dram_tensor("xt", [128, int(in_off[-1])], f16, kind="ExternalInput")
    out = nc.dram_tensor("out", [128, int(out_off[-1])], f16, kind="ExternalOutput")

    # Split each slot's blocks between DVE and ACT, balancing estimated ns
    # (DVE ~1.04 ns/row + ~130 ns/instr; ACT ~0.83 ns/row + ~150 ns/instr).
    # ACT starts ~1.6us late (the act-table load runs first).
    dve_blk, act_blk = [], []
    dc, ac = 0.0, 1600.0
    for bl in blocks:
        db, ab = [], []
        for blk in sorted(bl, key=lambda x: -x[1]):
            cd, ca = blk[1] * 1.04 + 130.0, blk[1] * 0.833 + 150.0
            if dc + cd <= ac + ca:
                db.append(blk)
                dc += cd
            else:
                ab.append(blk)
                ac += ca
        dve_blk.append(sorted(db))
        act_blk.append(sorted(ab))

    from contextlib import ExitStack

    with ExitStack() as st:
        xb = st.enter_context(nc.sbuf_tensor("xb", [128, int(in_off[-1])], f16))
        ob = st.enter_context(nc.sbuf_tensor("ob", [128, int(out_off[-1])], f16))
        wb = st.enter_context(nc.sbuf_tensor("wb", [128, 256], f16))
        ps = [
            st.enter_context(nc.psum_tensor(f"ps{i}", [128, 512], f32))
            for i in range(8)
        ]
        # one semaphore per input DMA chunk: every wait is exactly "this
        # DMA's own 16 completion increments" — cumulative thresholds
        # across different DMAs on one semaphore are NOT ordering-safe
        # (observed intermittent stale reads)
        in_sems = [
            st.enter_context(nc.semaphore(f"in_sem{i}")) for i in range(n_chunks)
        ]
        out_sem = st.enter_context(nc.semaphore("out_sem"))
        mm_sem = st.enter_context(nc.semaphore("mm_sem"))
        cp_sem = st.enter_context(nc.semaphore("cp_sem"))
        all_sems = in_sems + [out_sem, mm_sem, cp_sem]
        sem_nums = sorted(s.num for s in all_sems)
        lo, hi = sem_nums[0], sem_nums[-1] + 1
        assert sem_nums == list(range(lo, hi)), sem_nums

        # BassBlock used directly (not nc.Block) so we can skip the
        # Block-exit all-engine barrier: the runtime's own end-of-program
        # epilogue synchronizes engines anyway, and without our barrier an
        # early-finishing engine starts its epilogue immediately instead
        # of idling until the last output DMA lands.
        block = bass.BassBlock(nc, f"block_{nc.next_id()}")
        block.__enter__()

        # global chunk list: (sem index, slot, klo, khi)
        chunk_list = []
        for j in range(J):
            for ci, (klo, khi) in enumerate(in_chunks[j]):
                chunk_list.append((int(in_base[j]) + ci, j, klo, khi))

        def issue_chunk(engine, ci, j, klo, khi):
            L = slot_lens[j]
            o0 = int(in_off[j])
            engine.dma_start(
                xb[:, o0 + klo * L : o0 + khi * L],
                xt[:, o0 + klo * L : o0 + khi * L],
            ).then_inc(in_sems[ci], 16)

        # output DMA groups: slots are merged into a few large DMAs
        # (descriptors up to ~9KB) — the write path stalls between DMA
        # boundaries, so fewer/larger DMAs keep more engines busy. The
        # last two slots stay separate so the final transfer is small
        # and starts the moment the last copy lands. A group spans the
        # inter-slot pad columns (garbage bytes, host ignores them).
        out_groups = [list(range(0, J - 4)), [J - 4, J - 3], [J - 2], [J - 1]]

        def issue_out(engine, grp):
            jlo, jhi = grp[0], grp[-1]
            o0, o1 = int(out_off[jlo]), int(out_off[jhi]) + T[jhi]
            engine.wait_ge(cp_sem, int(cum_mb[jhi + 1]))
            engine.dma_start(
                out[:, o0:o1], ob[:, o0:o1]
            ).then_inc(out_sem, 16)

        @block.sync
        def _(sync):
            # ALL input chunks on this one queue, in PE need order.
            # Two interleaved read queues peak higher (~390 GB/s) but
            # the per-engine queue arbitration is bursty: one queue
            # regularly starves for 2-5us, the PE stalls mid-stream and
            # its clock ramps down (~3us of half-rate to recover). A
            # single queue is slower (~275 GB/s) but exactly in order
            # and jitter-free, which nets out faster end-to-end.
            for i, (ci, j, klo, khi) in enumerate(chunk_list):
                if i >= 6:
                    # cap in-flight DMAs on this ring (descriptor
                    # scratch is finite; unpaced issue wedges the
                    # exec unit)
                    sync.wait_ge(in_sems[chunk_list[i - 6][0]], 16)
                issue_chunk(sync, ci, j, klo, khi)


        @block.tensor
        def _(tensor):
            # warmup: junk matmuls on the (uninitialized) wb scratch while
            # the first input chunk is on the wire — keeps the PE clock
            # ramping so the real stream doesn't pay ~3us of half-rate.
            for _w in range(N_WARMUP):
                nc.tensor.matmul(
                    ps[4][:128, :256], wb[:, :128], wb[:, :256],
                    start=True, stop=True,
                )
            for j in range(J):
                L = slot_lens[j]
                o0 = int(in_off[j])
                if j >= 2:
                    # PSUM bank set j%2 free once slot j-2 fully copied out
                    tensor.wait_ge(cp_sem, int(cum_mb[j - 1]))
                pb = (j % 2) * 4
                for ci, (klo, khi) in enumerate(in_chunks[j]):
                    tensor.wait_ge(in_sems[int(in_base[j]) + ci], 16)
                    for m, W, rows, _t in blocks[j]:
                        for k in range(klo, khi):
                            o = o0 + k * L + 128 * m
                            lhsT = xb[:, o : o + rows]
                            rhs = xb[:, o : o0 + k * L + L]
                            ins = nc.tensor.matmul(
                                ps[pb + m][:rows, :W], lhsT, rhs,
                                start=(k == 0), stop=(k == KCH - 1),
                                skip_group_check=(len(in_chunks[j]) > 1),
                            )
                            if k == KCH - 1:
                                ins.then_inc(mm_sem, 1)

        @block.vector
        def _(vector):
            for j in range(J):
                pb = (j % 2) * 4
                o0 = int(out_off[j])
                for m, W, rows, t in dve_blk[j]:
                    vector.wait_ge(mm_sem, int(cum_mb[j]) + m + 1)
                    nc.vector.tensor_copy(
                        ob[:rows, o0 + t : o0 + t + W], ps[pb + m][:rows, :W]
                    ).then_inc(cp_sem, 1)

        @block.scalar
        def _(scalar):
            # pre-trigger the ACT table load for Copy off the critical path
            # (SBUF->SBUF dummy; slot 0's real copies overwrite this region)
            nc.scalar.copy(ob[:1, :8], wb[:1, :8])
            for j in range(J):
                pb = (j % 2) * 4
                o0 = int(out_off[j])
                for m, W, rows, t in act_blk[j]:
                    scalar.wait_ge(mm_sem, int(cum_mb[j]) + m + 1)
                    nc.scalar.copy(
                        ob[:rows, o0 + t : o0 + t + W], ps[pb + m][:rows, :W]
                    ).then_inc(cp_sem, 1)


        @block.gpsimd
        def _(g):
            # All output DMAs on the (otherwise idle) GpSimd SWDGE queue:
            # the HWDGE read queues starve it completely while they have
            # backlog, which is exactly the priority we want — writes
            # never steal engine time from the input stream the PE is
            # chasing, and the big grouped descriptors drain at full
            # wire rate (~385 GB/s) once the reads finish.
            for grp in out_groups:
                issue_out(g, grp)
            # Tail: wait for every semaphore's final value, then reset them
            # so re-executing the loaded NEFF stays correct (NRT does not
            # clear sems between executes).
            for s in in_sems:
                g.wait_ge(s, 16)
            g.wait_ge(out_sem, 16 * len(out_groups))
            g.wait_ge(mm_sem, int(cum_mb[J]))
            g.wait_ge(cp_sem, int(cum_mb[J]))
            g.dma_reset(range(lo, hi))
            g.sem_clear(range(lo, hi))

        # manual block exit minus the all-engine barrier
        for engine, last_body in block.last_body.items():
            with nc.body(last_body, parent=nc.cur_bb, allow_existing_parent=True):
                engine.br(block.end_bb)
        nc.switch_bb(block.end_bb)

    return nc


def _plan(graph_ids):
    """Sort graphs by length desc, deal round-robin: slot s of core c gets
    rank 8*s + c. slot_lens[s] = max length within the slot = rank 8*s."""
    lengths = np.bincount(graph_ids, minlength=B)
    order = np.argsort(-lengths, kind="stable")  # rank -> graph id
    # round up to multiple of 32 so every fp16 matmul operand byte offset
    # (k*L*2) stays 64B-aligned (walrus ISA check on matmul operands)
    slot_lens = tuple(
        (int(lengths[order[N_CORES * s]]) + 31) // 32 * 32 for s in range(PER_CORE)
    )
    assign = order.reshape(PER_CORE, N_CORES)  # [slot, core] -> graph id
    return lengths, assign, slot_lens


def _get_program(slot_lens):
    if slot_lens not in _prog_cache:
        _prog_cache[slot_lens] = _build_program(list(slot_lens))
    return _prog_cache[slot_lens]


def _host_prepare(batched_h, graph_ids, pos_ids, lengths, assign, slot_lens):
    """Build per-core [128, sum(KCH*L_s)] fp16 inputs:
    xt[p, in_off[s] + k*L_s + n] = h[g_cs][node n][d=128k+p]."""
    padded = np.zeros((B, MAXN, D), dtype=np.float16)
    padded[graph_ids, pos_ids] = batched_h.astype(np.float16)
    in_w = sum(KCH * l for l in slot_lens)
    ins = []
    for c in range(N_CORES):
        a = np.zeros((128, in_w), dtype=np.float16)
        off = 0
        for s, L in enumerate(slot_lens):
            g = assign[s, c]
            n = int(lengths[g])
            # [n, D] -> [D, n] -> [KCH, 128, n] -> [128, KCH, n]
            xtg = padded[g, :n].T.reshape(KCH, 128, n).transpose(1, 0, 2)
            blk = a[:, off : off + KCH * L].reshape(128, KCH, L)
            blk[:, :, :n] = xtg
            off += KCH * L
        ins.append(a)
    return ins


def _host_gather(results, lengths, assign, slot_lens):
    """Paste upper-triangular blocks and mirror the strictly-lower part.
    Padded rows/cols beyond each graph's n hold exact zeros (zero-padded
    input rows), matching the reference output, so no cropping is needed."""
    full = np.zeros((B, MAXN, MAXN), dtype=np.float32)
    for c in range(N_CORES):
        o = results[c]["out"]
        off = 0
        for s, L in enumerate(slot_lens):
            g = assign[s, c]
            bl = _slot_blocks(L)
            for m, W, rows, t in bl:
                r0 = 128 * m
                blk = o[:rows, off + t : off + t + W].astype(np.float32)
                full[g, r0 : r0 + rows, r0 : r0 + W] = blk
                if W > rows:
                    full[g, r0 + rows : r0 + W, r0 : r0 + rows] = blk[:, rows:].T
            # slot offsets padded to 128 elems (DMA write alignment)
            off += (bl[-1][3] + bl[-1][1] + 127) // 128 * 128
    return full


def kernel(batched_h, graph_ids, pos_ids, B=None, max_n=None, **_ignored):
    from concourse.bass_utils import run_bass_kernel_spmd

    batched_h = np.asarray(batched_h, dtype=np.float32)
    graph_ids = np.asarray(graph_ids, dtype=np.int64)
    pos_ids = np.asarray(pos_ids, dtype=np.int64)

    lengths, assign, slot_lens = _plan(graph_ids)
    nc = _get_program(slot_lens)
    in_maps = [
        {"xt": a}
        for a in _host_prepare(batched_h, graph_ids, pos_ids, lengths, assign, slot_lens)
    ]
    res = run_bass_kernel_spmd(nc, in_maps, list(range(N_CORES)))
    return _host_gather(res.results, lengths, assign, slot_lens)
